# revision 26
# baseline (speedup 1.0000x reference)
"""Trainium2 Bass kernel for a circular-padded 3x3 conv cellular-automaton step.

Computation (per image):
    z   = conv3x3_circular(x, Wc) ;  Wc = w1 @ w_perc  (host-fused, [96,12,3,3])
    h   = relu(z + b1)
    u   = w2 @ h + b2
    out = x + (mask > 0.5) * u

Mapping (per core, B=16 split 8 ways -> 2 images/core):
  * conv as two accumulating matmuls per image row:
      K=72  (partitions (dj,di,c): row-shifts di loaded 3x from DRAM, column
             shift dj=1 produced by one on-chip offset copy)
      K=36  (dj=2 read from the dj=0 partition block at free offset +2)
    weights bf16, activations bf16, fp32 PSUM accumulate.
  * relu+bias split across ScalarE (2 rows) and VectorE (2 rows) per 4-row supertile,
    output bf16 `h` with a constant ones-row (row 96) so b2 rides in the matmul.
  * second matmul (w2 [12,96] zero-padded to [97,32], b2 in row 96) as 4 concurrent
    column-group matmuls -> one full [128, 384] PSUM update tile per supertile.
  * mask multiply on VectorE against a host-prelaid [128, 96*384] bf16 mask tensor,
    +x on GPSIMD (x arrives host-prearranged, zero-padded to 128 partitions), output
    staged per chunk and stored with one padded DMA; host inverse-permutes.
  * DMA instruction count is the enemy (~1.3us fixed cost each): 4 per chunk.
"""

import sys

if "/opt/trn_rl_repo" not in sys.path:
    sys.path.insert(0, "/opt/trn_rl_repo")

from contextlib import ExitStack

import numpy as np
import ml_dtypes

import concourse.bass as bass
import concourse.tile as tile
from concourse import mybir
from concourse.bass_utils import run_bass_kernel_spmd

B, C, H, W = 16, 12, 384, 384
CH = 96                      # hidden channels
NCORES = 8
BLOC = B // NCORES           # images per core
W2 = W + 2                   # circular-padded row length
PADH = H + 4                 # padded rows: 1 top + 3 bottom
CHUNK = 32                   # image rows per processing chunk
ST = 4                       # rows per supertile (one per PE column group)
NCHUNK = H // CHUNK
NST = CHUNK // ST            # supertiles per chunk
XQLEN = CHUNK * W2           # used free length per chunk
XBLEN = (CHUNK + 2) * W2     # loaded free length (halo + shift spill)
MTILES = H // ST             # 96 supertile row-groups per image
STW = NST * W                # supertile-layout free length per chunk

_BF16 = mybir.dt.bfloat16
_F32 = mybir.dt.float32


def _spill_waits(nc):
    """walrus/trn2 here accepts at most ONE sync-wait per instruction; move
    excess waits onto NoOps inserted immediately before, on the same engine."""
    nspill = 0
    for bbwrap in list(nc.bb_map.values()):
        bb = bbwrap.bb
        out = []
        for inst in bb.instructions:
            si = inst.sync_info
            if si is not None and si.on_wait and len(si.on_wait) > 1:
                waits = list(si.on_wait)
                for w in waits[1:]:
                    nop = mybir.InstNoOp(
                        name=nc.get_next_instruction_name(),
                        engine=inst.engine,
                        sync_info=mybir.SyncInfo(on_wait=[w], on_update=[]),
                        bass_nofuse=True,
                    )
                    nc.register_instruction(nop)
                    out.append(nop)
                    nspill += 1
                si.on_wait = waits[:1]
            out.append(inst)
        try:
            bb.instructions = out
        except Exception:
            bb.instructions.clear()
            bb.instructions.extend(out)
    return nspill


def _build_nc(reps=1):
    nc = bass.Bass()

    xpad = nc.declare_dram_parameter("xpad", [BLOC, C, PADH, W2], _BF16, isOutput=False)
    xst = nc.declare_dram_parameter("xst", [BLOC, NCHUNK, 128, STW], _BF16, isOutput=False)
    wa = nc.declare_dram_parameter("wa", [72, CH], _BF16, isOutput=False)
    wb = nc.declare_dram_parameter("wb", [36, CH], _BF16, isOutput=False)
    w2p = nc.declare_dram_parameter("w2p", [CH + 1, 32], _BF16, isOutput=False)
    b1 = nc.declare_dram_parameter("b1", [CH, 1], _F32, isOutput=False)
    m128 = nc.declare_dram_parameter("m128", [128, MTILES * W], _BF16, isOutput=False)
    out = nc.declare_dram_parameter("out", [BLOC, NCHUNK, 128, STW], _F32, isOutput=True)

    with tile.TileContext(nc) as tc, ExitStack() as ctx:
        state = _setup(ctx, tc, wa, wb, w2p, b1, m128)
        if reps == 1:
            _loop_body(tc, state, xpad, xst, out)
        else:
            # benchmark mode: repeat the whole computation on-device so the
            # per-call dispatch overhead (~100ms via axon) amortizes away
            with tc.For_i(0, reps, 1):
                _loop_body(tc, state, xpad, xst, out)
    _spill_waits(nc)
    return nc


def _setup(ctx, tc, wa, wb, w2p, b1, m128):
    nc = tc.nc

    const = ctx.enter_context(tc.tile_pool(name="const", bufs=1))
    ump = ctx.enter_context(tc.tile_pool(name="um", bufs=2))
    zp = ctx.enter_context(tc.tile_pool(name="z", bufs=3, space="PSUM"))
    up = ctx.enter_context(tc.tile_pool(name="u", bufs=2, space="PSUM"))

    wa_sb = const.tile([72, CH], _BF16)
    nc.sync.dma_start(out=wa_sb, in_=wa[:, :])
    wb_sb = const.tile([36, CH], _BF16)
    nc.sync.dma_start(out=wb_sb, in_=wb[:, :])
    w2p_sb = const.tile([CH + 1, 32], _BF16)
    nc.sync.dma_start(out=w2p_sb, in_=w2p[:, :])
    b1_sb = const.tile([CH, 1], _F32)
    nc.sync.dma_start(out=b1_sb, in_=b1[:, :])
    m128_sb = const.tile([128, MTILES * W], _BF16)
    nc.sync.dma_start(out=m128_sb, in_=m128[:, :])

    # manually double-buffered tiles (stable addresses):
    #  - ht: constant ones-row (row 96) carries b2 through the second matmul
    #  - xq: conv input, partitions (dj 0..1, di 0..2, c); memset once
    #  - xt: x in supertile layout (host pre-padded to 128 partitions)
    #  - ot: output staging in supertile layout
    hts = [
        const.tile([CH + 1, ST, W], _BF16, name=f"ht{i}", tag=f"ht{i}")
        for i in range(2)
    ]
    xqs = [
        const.tile([72, XBLEN], _BF16, name=f"xqt{i}", tag=f"xqt{i}") for i in range(2)
    ]
    xts = [
        const.tile([128, STW], _BF16, name=f"xtt{i}", tag=f"xtt{i}") for i in range(2)
    ]
    ots = [
        const.tile([128, STW], _F32, name=f"ott{i}", tag=f"ott{i}") for i in range(2)
    ]
    for t in hts + xqs + xts + ots:
        nc.vector.memset(t, 0.0)
    for ht in hts:
        nc.vector.memset(ht[CH : CH + 1, :, :], 1.0)

    # warmup matmuls: absorb the weight-load DMA waits on the PE clock so the
    # first real matmul of a chunk only waits on its own input DMAs (the HW
    # allows a single sync-wait per instruction; extras become NoOps)
    zw = zp.tile([CH, 2, 512], _F32, tag="z2")
    nc.tensor.matmul(zw[:, 0, 0:1], wa_sb, xqs[0][:, 0:1], start=True, stop=True)
    nc.tensor.matmul(zw[:, 1, 0:1], wb_sb, xqs[0][0:36, 0:1], start=True, stop=True)
    uw = up.tile([128, W], _F32, tag="u")
    nc.tensor.matmul(
        uw[0:32, 0:1], w2p_sb, hts[0][:, 0, 0:1], start=True, stop=True,
        tile_position=(0, 0),
    )

    return dict(
        ump=ump, zp=zp, up=up,
        wa_sb=wa_sb, wb_sb=wb_sb, w2p_sb=w2p_sb, b1_sb=b1_sb, m128_sb=m128_sb,
        hts=hts, xqs=xqs, xts=xts, ots=ots,
    )


def _loop_body(tc, state, xpad, xst, out):
    nc = tc.nc
    add = mybir.AluOpType.add
    mult = mybir.AluOpType.mult
    amax = mybir.AluOpType.max
    relu = mybir.ActivationFunctionType.Relu
    ump, zp, up = state["ump"], state["zp"], state["up"]
    wa_sb, wb_sb, w2p_sb, b1_sb, m128_sb = (
        state["wa_sb"], state["wb_sb"], state["w2p_sb"],
        state["b1_sb"], state["m128_sb"],
    )
    hts, xqs, xts, ots = state["hts"], state["xqs"], state["xts"], state["ots"]

    nbuf = 0
    ncbuf = 0
    for b in range(BLOC):
        for chk in range(NCHUNK):
            r0 = chk * CHUNK
            xq = xqs[ncbuf % 2]
            xt = xts[ncbuf % 2]
            ot = ots[ncbuf % 2]
            ncbuf += 1

            # one DMA: partitions 12*di+c <- x rows r0-1+di .. (wrap-padded),
            # i.e. the three row-shifted copies, straight from DRAM
            src = bass.AP(
                tensor=xpad,
                offset=(b * C * PADH + r0) * W2,
                ap=[[W2, 3], [PADH * W2, C], [1, XBLEN]],
            )
            nc.sync.dma_start(out=xq[0:36, :], in_=src)
            # one on-chip copy: partitions 36..71 = same data at +1 column
            nc.sync.dma_start(
                out=xq[36:72, 0 : XQLEN + W], in_=xq[0:36, 1 : 1 + XQLEN + W]
            )
            # x in supertile layout (pad partitions pre-zeroed host-side)
            nc.sync.dma_start(out=xt, in_=xst[b, chk, :, :])

            for st in range(NST):
                z2a = zp.tile([CH, 2, 512], _F32, tag="z2")
                z2b = zp.tile([CH, 2, 512], _F32, tag="z2")
                for j in range(ST):
                    q = st * ST + j
                    zt = (z2a if j < 2 else z2b)[:, j % 2, 0:W]
                    nc.tensor.matmul(
                        zt,
                        wa_sb,
                        xq[0:72, q * W2 : q * W2 + W],
                        start=True,
                        stop=False,
                    )
                    nc.tensor.matmul(
                        zt,
                        wb_sb,
                        xq[0:36, q * W2 + 2 : q * W2 + 2 + W],
                        start=False,
                        stop=True,
                    )

                ht = hts[nbuf % 2]
                nc.scalar.activation(
                    out=ht[0:CH, 0:2, :], in_=z2a[:, :, 0:W], func=relu, bias=b1_sb
                )
                nc.vector.tensor_scalar(
                    ht[0:CH, 2:4, :], z2b[:, :, 0:W], b1_sb, 0.0, add, amax
                )

                u = up.tile([128, W], _F32, tag="u")
                for j in range(ST):
                    nc.tensor.matmul(
                        u[32 * j : 32 * j + 32, :],
                        w2p_sb,
                        ht[:, j, :],
                        start=True,
                        stop=True,
                        tile_position=(0, 32 * j),
                    )

                tglob = chk * NST + st
                um = ump.tile([128, W], _BF16)
                nc.vector.tensor_tensor(
                    um, u, m128_sb[:, tglob * W : tglob * W + W], mult
                )
                nc.gpsimd.tensor_tensor(
                    ot[:, st * W : st * W + W], um, xt[:, st * W : st * W + W], add
                )
                nbuf += 1

            nc.sync.dma_start(out=out[b, chk, :, :], in_=ot)


_NC_CACHE = {}


def _get_nc():
    if "nc" not in _NC_CACHE:
        _NC_CACHE["nc"] = _build_nc()
    return _NC_CACHE["nc"]


def _prep_inputs(x, w_perc, w1, b1, w2, b2, mask):
    bf16 = ml_dtypes.bfloat16
    wc = np.einsum("hp,pcij->hcij", w1, w_perc).astype(np.float32)  # [96,12,3,3]
    # wa[12*di+c + 36*dj, h] = wc[h, c, di, dj] for dj in {0,1}
    wdjdic = wc.transpose(3, 2, 1, 0)  # [dj, di, c, h]
    wa = np.ascontiguousarray(wdjdic[0:2].reshape(72, CH)).astype(bf16)
    wb = np.ascontiguousarray(wdjdic[2].reshape(36, CH)).astype(bf16)
    w2p = np.zeros((CH + 1, 32), np.float32)
    w2p[0:CH, 0:C] = w2.T
    w2p[CH, 0:C] = b2
    w2p = w2p.astype(bf16)
    b1c = np.ascontiguousarray(b1.reshape(CH, 1)).astype(np.float32)

    mbit = (mask > 0.5).astype(np.float32)
    m128 = np.zeros((128, MTILES * W), np.float32)
    for j in range(ST):
        rows = mbit[j::ST, :].reshape(MTILES * W)
        for c in range(C):
            m128[32 * j + c] = rows
    m128 = m128.astype(bf16)

    xb16 = x.astype(bf16)
    in_maps = []
    for core in range(NCORES):
        xs = np.ascontiguousarray(x[core * BLOC : (core + 1) * BLOC], np.float32)
        xsp = np.pad(
            xb16[core * BLOC : (core + 1) * BLOC],
            ((0, 0), (0, 0), (1, 3), (1, 1)),
            mode="wrap",
        )
        # supertile layout: xst[b, chk, 32*j+c, s*W+w] = x[b, c, 16*chk+4*s+j, w]
        xst = np.zeros((BLOC, NCHUNK, 4, 32, NST, W), np.float32)
        xr = xs.reshape(BLOC, C, NCHUNK, NST, ST, W).transpose(0, 2, 4, 1, 3, 5)
        xst[:, :, :, 0:C] = xr
        xst = xst.reshape(BLOC, NCHUNK, 128, STW).astype(bf16)
        in_maps.append(
            {
                "xpad": np.ascontiguousarray(xsp),
                "xst": np.ascontiguousarray(xst),
                "wa": wa,
                "wb": wb,
                "w2p": w2p,
                "b1": b1c,
                "m128": m128,
            }
        )
    return in_maps


def _unshard_out(core_outs):
    full = np.empty((B, C, H, W), np.float32)
    for core, o in enumerate(core_outs):
        o = np.asarray(o, np.float32).reshape(BLOC, NCHUNK, ST, 32, NST, W)
        o = o[:, :, :, 0:C]  # drop pad partitions
        # [b, chk, j, c, s, w] -> [b, c, (chk s j), w]
        o = o.transpose(0, 3, 1, 4, 2, 5).reshape(BLOC, C, H, W)
        full[core * BLOC : (core + 1) * BLOC] = o
    return full


def kernel(x, w_perc, w1, b1, w2, b2, mask):
    x = np.asarray(x, dtype=np.float32)
    in_maps = _prep_inputs(
        x,
        np.asarray(w_perc, np.float32),
        np.asarray(w1, np.float32),
        np.asarray(b1, np.float32),
        np.asarray(w2, np.float32),
        np.asarray(b2, np.float32),
        np.asarray(mask, np.float32),
    )
    nc = _get_nc()
    res = run_bass_kernel_spmd(nc, in_maps, core_ids=list(range(NCORES)))
    return _unshard_out([r["out"] for r in res.results])


# revision 27
# speedup vs baseline: 1.0022x; 1.0022x over previous
"""Trainium2 Bass kernel for a circular-padded 3x3 conv cellular-automaton step.

Computation (per image):
    z   = conv3x3_circular(x, Wc) ;  Wc = w1 @ w_perc  (host-fused, [96,12,3,3])
    h   = relu(z + b1)
    u   = w2 @ h + b2
    out = x + (mask > 0.5) * u

Mapping (per core, B=16 split 8 ways -> 2 images/core):
  * conv as two accumulating matmuls per image row:
      K=72  (partitions (dj,di,c): row-shifts di loaded 3x from DRAM, column
             shift dj=1 produced by one on-chip offset copy)
      K=36  (dj=2 read from the dj=0 partition block at free offset +2)
    weights bf16, activations bf16, fp32 PSUM accumulate.
  * relu+bias split across ScalarE (2 rows) and VectorE (2 rows) per 4-row supertile,
    output bf16 `h` with a constant ones-row (row 96) so b2 rides in the matmul.
  * second matmul (w2 [12,96] zero-padded to [97,32], b2 in row 96) as 4 concurrent
    column-group matmuls -> one full [128, 384] PSUM update tile per supertile.
  * mask multiply on VectorE against a host-prelaid [128, 96*384] bf16 mask tensor,
    +x on GPSIMD (x arrives host-prearranged, zero-padded to 128 partitions), output
    staged per chunk and stored with one padded DMA; host inverse-permutes.
  * DMA instruction count is the enemy (~1.3us fixed cost each): 4 per chunk.
"""

import sys

if "/opt/trn_rl_repo" not in sys.path:
    sys.path.insert(0, "/opt/trn_rl_repo")

from contextlib import ExitStack

import numpy as np
import ml_dtypes

import concourse.bass as bass
import concourse.tile as tile
from concourse import mybir
from concourse.bass_utils import run_bass_kernel_spmd

B, C, H, W = 16, 12, 384, 384
CH = 96                      # hidden channels
NCORES = 8
BLOC = B // NCORES           # images per core
W2 = W + 2                   # circular-padded row length
PADH = H + 4                 # padded rows: 1 top + 3 bottom
CHUNK = 16                   # image rows per processing chunk
ST = 4                       # rows per supertile (one per PE column group)
NCHUNK = H // CHUNK
NST = CHUNK // ST            # supertiles per chunk
XQLEN = CHUNK * W2           # used free length per chunk
XBLEN = (CHUNK + 2) * W2     # loaded free length (halo + shift spill)
MTILES = H // ST             # 96 supertile row-groups per image
STW = NST * W                # supertile-layout free length per chunk

_BF16 = mybir.dt.bfloat16
_F32 = mybir.dt.float32


def _spill_waits(nc):
    """walrus/trn2 here accepts at most ONE sync-wait per instruction; move
    excess waits onto NoOps inserted immediately before, on the same engine."""
    nspill = 0
    for bbwrap in list(nc.bb_map.values()):
        bb = bbwrap.bb
        out = []
        for inst in bb.instructions:
            si = inst.sync_info
            if si is not None and si.on_wait and len(si.on_wait) > 1:
                waits = list(si.on_wait)
                for w in waits[1:]:
                    nop = mybir.InstNoOp(
                        name=nc.get_next_instruction_name(),
                        engine=inst.engine,
                        sync_info=mybir.SyncInfo(on_wait=[w], on_update=[]),
                        bass_nofuse=True,
                    )
                    nc.register_instruction(nop)
                    out.append(nop)
                    nspill += 1
                si.on_wait = waits[:1]
            out.append(inst)
        try:
            bb.instructions = out
        except Exception:
            bb.instructions.clear()
            bb.instructions.extend(out)
    return nspill


def _build_nc(reps=1):
    nc = bass.Bass()

    xpad = nc.declare_dram_parameter("xpad", [BLOC, C, PADH, W2], _BF16, isOutput=False)
    xst = nc.declare_dram_parameter("xst", [BLOC, NCHUNK, 128, STW], _BF16, isOutput=False)
    wa = nc.declare_dram_parameter("wa", [72, CH], _BF16, isOutput=False)
    wb = nc.declare_dram_parameter("wb", [36, CH], _BF16, isOutput=False)
    w2p = nc.declare_dram_parameter("w2p", [CH + 1, 32], _BF16, isOutput=False)
    b1 = nc.declare_dram_parameter("b1", [CH, 1], _F32, isOutput=False)
    m128 = nc.declare_dram_parameter("m128", [128, MTILES * W], _BF16, isOutput=False)
    out = nc.declare_dram_parameter("out", [BLOC, NCHUNK, 128, STW], _F32, isOutput=True)

    with tile.TileContext(nc) as tc, ExitStack() as ctx:
        state = _setup(ctx, tc, wa, wb, w2p, b1, m128)
        if reps == 1:
            _loop_body(tc, state, xpad, xst, out)
        else:
            # benchmark mode: repeat the whole computation on-device so the
            # per-call dispatch overhead (~100ms via axon) amortizes away
            with tc.For_i(0, reps, 1):
                _loop_body(tc, state, xpad, xst, out)
    _spill_waits(nc)
    return nc


def _setup(ctx, tc, wa, wb, w2p, b1, m128):
    nc = tc.nc

    const = ctx.enter_context(tc.tile_pool(name="const", bufs=1))
    ump = ctx.enter_context(tc.tile_pool(name="um", bufs=2))
    zp = ctx.enter_context(tc.tile_pool(name="z", bufs=3, space="PSUM"))
    up = ctx.enter_context(tc.tile_pool(name="u", bufs=2, space="PSUM"))

    wa_sb = const.tile([72, CH], _BF16)
    nc.sync.dma_start(out=wa_sb, in_=wa[:, :])
    wb_sb = const.tile([36, CH], _BF16)
    nc.sync.dma_start(out=wb_sb, in_=wb[:, :])
    w2p_sb = const.tile([CH + 1, 32], _BF16)
    nc.sync.dma_start(out=w2p_sb, in_=w2p[:, :])
    b1_sb = const.tile([CH, 1], _F32)
    nc.sync.dma_start(out=b1_sb, in_=b1[:, :])
    m128_sb = const.tile([128, MTILES * W], _BF16)
    nc.sync.dma_start(out=m128_sb, in_=m128[:, :])

    # manually double-buffered tiles (stable addresses):
    #  - ht: constant ones-row (row 96) carries b2 through the second matmul
    #  - xq: conv input, partitions (dj 0..1, di 0..2, c); memset once
    #  - xt: x in supertile layout (host pre-padded to 128 partitions)
    #  - ot: output staging in supertile layout
    hts = [
        const.tile([CH + 1, ST, W], _BF16, name=f"ht{i}", tag=f"ht{i}")
        for i in range(2)
    ]
    xqs = [
        const.tile([72, XBLEN], _BF16, name=f"xqt{i}", tag=f"xqt{i}") for i in range(2)
    ]
    xts = [
        const.tile([128, STW], _BF16, name=f"xtt{i}", tag=f"xtt{i}") for i in range(2)
    ]
    ots = [
        const.tile([128, STW], _F32, name=f"ott{i}", tag=f"ott{i}") for i in range(2)
    ]
    for t in hts + xqs + xts + ots:
        nc.vector.memset(t, 0.0)
    for ht in hts:
        nc.vector.memset(ht[CH : CH + 1, :, :], 1.0)

    # warmup matmuls: absorb the weight-load DMA waits on the PE clock so the
    # first real matmul of a chunk only waits on its own input DMAs (the HW
    # allows a single sync-wait per instruction; extras become NoOps)
    zw = zp.tile([CH, 2, 512], _F32, tag="z2")
    nc.tensor.matmul(zw[:, 0, 0:1], wa_sb, xqs[0][:, 0:1], start=True, stop=True)
    nc.tensor.matmul(zw[:, 1, 0:1], wb_sb, xqs[0][0:36, 0:1], start=True, stop=True)
    uw = up.tile([128, W], _F32, tag="u")
    nc.tensor.matmul(
        uw[0:32, 0:1], w2p_sb, hts[0][:, 0, 0:1], start=True, stop=True,
        tile_position=(0, 0),
    )

    return dict(
        ump=ump, zp=zp, up=up,
        wa_sb=wa_sb, wb_sb=wb_sb, w2p_sb=w2p_sb, b1_sb=b1_sb, m128_sb=m128_sb,
        hts=hts, xqs=xqs, xts=xts, ots=ots,
    )


def _loop_body(tc, state, xpad, xst, out):
    nc = tc.nc
    add = mybir.AluOpType.add
    mult = mybir.AluOpType.mult
    amax = mybir.AluOpType.max
    relu = mybir.ActivationFunctionType.Relu
    ump, zp, up = state["ump"], state["zp"], state["up"]
    wa_sb, wb_sb, w2p_sb, b1_sb, m128_sb = (
        state["wa_sb"], state["wb_sb"], state["w2p_sb"],
        state["b1_sb"], state["m128_sb"],
    )
    hts, xqs, xts, ots = state["hts"], state["xqs"], state["xts"], state["ots"]

    nbuf = 0
    ncbuf = 0
    for b in range(BLOC):
        for chk in range(NCHUNK):
            r0 = chk * CHUNK
            xq = xqs[ncbuf % 2]
            xt = xts[ncbuf % 2]
            ot = ots[ncbuf % 2]
            ncbuf += 1

            # one DMA: partitions 12*di+c <- x rows r0-1+di .. (wrap-padded),
            # i.e. the three row-shifted copies, straight from DRAM
            src = bass.AP(
                tensor=xpad,
                offset=(b * C * PADH + r0) * W2,
                ap=[[W2, 3], [PADH * W2, C], [1, XBLEN]],
            )
            nc.sync.dma_start(out=xq[0:36, :], in_=src)
            # one on-chip copy: partitions 36..71 = same data at +1 column
            nc.sync.dma_start(
                out=xq[36:72, 0 : XQLEN + W], in_=xq[0:36, 1 : 1 + XQLEN + W]
            )
            # x in supertile layout (pad partitions pre-zeroed host-side)
            nc.sync.dma_start(out=xt, in_=xst[b, chk, :, :])

            for st in range(NST):
                z2a = zp.tile([CH, 2, 512], _F32, tag="z2")
                z2b = zp.tile([CH, 2, 512], _F32, tag="z2")
                for j in range(ST):
                    q = st * ST + j
                    zt = (z2a if j < 2 else z2b)[:, j % 2, 0:W]
                    nc.tensor.matmul(
                        zt,
                        wa_sb,
                        xq[0:72, q * W2 : q * W2 + W],
                        start=True,
                        stop=False,
                    )
                    nc.tensor.matmul(
                        zt,
                        wb_sb,
                        xq[0:36, q * W2 + 2 : q * W2 + 2 + W],
                        start=False,
                        stop=True,
                    )

                ht = hts[nbuf % 2]
                nc.scalar.activation(
                    out=ht[0:CH, 0:2, :], in_=z2a[:, :, 0:W], func=relu, bias=b1_sb
                )
                nc.vector.tensor_scalar(
                    ht[0:CH, 2:4, :], z2b[:, :, 0:W], b1_sb, 0.0, add, amax
                )

                u = up.tile([128, W], _F32, tag="u")
                for j in range(ST):
                    nc.tensor.matmul(
                        u[32 * j : 32 * j + 32, :],
                        w2p_sb,
                        ht[:, j, :],
                        start=True,
                        stop=True,
                        tile_position=(0, 32 * j),
                    )

                tglob = chk * NST + st
                um = ump.tile([128, W], _BF16)
                nc.vector.tensor_tensor(
                    um, u, m128_sb[:, tglob * W : tglob * W + W], mult
                )
                nc.gpsimd.tensor_tensor(
                    ot[:, st * W : st * W + W], um, xt[:, st * W : st * W + W], add
                )
                nbuf += 1

            nc.sync.dma_start(out=out[b, chk, :, :], in_=ot)


_NC_CACHE = {}


def _get_nc():
    if "nc" not in _NC_CACHE:
        _NC_CACHE["nc"] = _build_nc()
    return _NC_CACHE["nc"]


def _prep_inputs(x, w_perc, w1, b1, w2, b2, mask):
    bf16 = ml_dtypes.bfloat16
    wc = np.einsum("hp,pcij->hcij", w1, w_perc).astype(np.float32)  # [96,12,3,3]
    # wa[12*di+c + 36*dj, h] = wc[h, c, di, dj] for dj in {0,1}
    wdjdic = wc.transpose(3, 2, 1, 0)  # [dj, di, c, h]
    wa = np.ascontiguousarray(wdjdic[0:2].reshape(72, CH)).astype(bf16)
    wb = np.ascontiguousarray(wdjdic[2].reshape(36, CH)).astype(bf16)
    w2p = np.zeros((CH + 1, 32), np.float32)
    w2p[0:CH, 0:C] = w2.T
    w2p[CH, 0:C] = b2
    w2p = w2p.astype(bf16)
    b1c = np.ascontiguousarray(b1.reshape(CH, 1)).astype(np.float32)

    mbit = (mask > 0.5).astype(np.float32)
    m128 = np.zeros((128, MTILES * W), np.float32)
    for j in range(ST):
        rows = mbit[j::ST, :].reshape(MTILES * W)
        for c in range(C):
            m128[32 * j + c] = rows
    m128 = m128.astype(bf16)

    xb16 = x.astype(bf16)
    in_maps = []
    for core in range(NCORES):
        xs = np.ascontiguousarray(x[core * BLOC : (core + 1) * BLOC], np.float32)
        xsp = np.pad(
            xb16[core * BLOC : (core + 1) * BLOC],
            ((0, 0), (0, 0), (1, 3), (1, 1)),
            mode="wrap",
        )
        # supertile layout: xst[b, chk, 32*j+c, s*W+w] = x[b, c, 16*chk+4*s+j, w]
        xst = np.zeros((BLOC, NCHUNK, 4, 32, NST, W), np.float32)
        xr = xs.reshape(BLOC, C, NCHUNK, NST, ST, W).transpose(0, 2, 4, 1, 3, 5)
        xst[:, :, :, 0:C] = xr
        xst = xst.reshape(BLOC, NCHUNK, 128, STW).astype(bf16)
        in_maps.append(
            {
                "xpad": np.ascontiguousarray(xsp),
                "xst": np.ascontiguousarray(xst),
                "wa": wa,
                "wb": wb,
                "w2p": w2p,
                "b1": b1c,
                "m128": m128,
            }
        )
    return in_maps


def _unshard_out(core_outs):
    full = np.empty((B, C, H, W), np.float32)
    for core, o in enumerate(core_outs):
        o = np.asarray(o, np.float32).reshape(BLOC, NCHUNK, ST, 32, NST, W)
        o = o[:, :, :, 0:C]  # drop pad partitions
        # [b, chk, j, c, s, w] -> [b, c, (chk s j), w]
        o = o.transpose(0, 3, 1, 4, 2, 5).reshape(BLOC, C, H, W)
        full[core * BLOC : (core + 1) * BLOC] = o
    return full


def kernel(x, w_perc, w1, b1, w2, b2, mask):
    x = np.asarray(x, dtype=np.float32)
    in_maps = _prep_inputs(
        x,
        np.asarray(w_perc, np.float32),
        np.asarray(w1, np.float32),
        np.asarray(b1, np.float32),
        np.asarray(w2, np.float32),
        np.asarray(b2, np.float32),
        np.asarray(mask, np.float32),
    )
    nc = _get_nc()
    res = run_bass_kernel_spmd(nc, in_maps, core_ids=list(range(NCORES)))
    return _unshard_out([r["out"] for r in res.results])
